# revision 17
# baseline (speedup 1.0000x reference)
"""MultiHeadAttention (dense, B=4 S=2048 D=1024 H=16) + residual + LayerNorm
on 8 Trainium2 NeuronCores.

Sharding: core c handles batch b=c//2 and head group g=c%2 (8 of 16 heads),
all 2048 query tokens. The output projection is row-parallel over d_in; a
pair-local bf16 ReduceScatter (cores 2b, 2b+1) sums the two half-head partial
fc outputs per 512-token chunk (4 chunks), after which each core does
bias+residual+LayerNorm for its 256-token half of each chunk (1024 total).

Device layouts (per core):
  QT/KT: [128 part, 4 chunk, 2048 tok] bf16  = projected Q^T / K^T (d_out on
         partitions; head h lives at chunk h//2, partitions 64*(h%2)..+64)
  VO:    [128 part, 16 kchunk, 8 head, 65] bf16 = V rows with a ones column
         appended (col 64) so att@V also yields softmax denominators
  scores are computed transposed ([k, q]) so exp output feeds att@V directly;
  att@V runs V-stationary producing att_out^T [65, q] whose row 64 is the
  softmax sum; normalization multiplies by a broadcast reciprocal.

Pipeline: V-projection runs kc-outer waves over 8 PSUM banks so the PE
starts as soon as the first XV/WV chunks land; per-hp normalization and the
LayerNorm epilogues of finished ReduceScatter chunks are interleaved into
later attention hp iterations so the tensor engine never waits on them.
"""

import numpy as np
import ml_dtypes

import concourse.bass as bass
import concourse.mybir as mybir
import concourse.tile as tile
from concourse import bacc
from concourse.bass_utils import run_bass_kernel_spmd

BF16 = mybir.dt.bfloat16
F32 = mybir.dt.float32
AF = mybir.ActivationFunctionType
OP = mybir.AluOpType

B = 4
S = 2048  # sequence length
D = 1024  # d_model
HL = 8  # heads per core
DK = 64  # head dim
DH = HL * DK  # 512 local projection width
P = 128
KC = S // P  # 16 key chunks
QC = 2  # q halves of 1024
NCH = 4  # reduce-scatter chunks of 512 tokens
CH = S // NCH  # 512
LN_EPS = 1e-5
SCALE = 1.0 / 8.0  # 1/sqrt(DK)
NRING = 8  # exp ring depth (in kc steps)

_NC_CACHE = None
_LAST_RES = None


def build_nc():
    nc = bacc.Bacc(
        None, target_bir_lowering=False, num_devices=8, dynamic_dma_scratch_size=2048
    )

    xqT = nc.declare_dram_parameter("xqT", [D, S], BF16, isOutput=False)
    xkT = nc.declare_dram_parameter("xkT", [D, S], BF16, isOutput=False)
    xvT = nc.declare_dram_parameter("xvT", [D, S], BF16, isOutput=False)
    wqT = nc.declare_dram_parameter("wqT", [D, DH], BF16, isOutput=False)
    wkT = nc.declare_dram_parameter("wkT", [D, DH], BF16, isOutput=False)
    wvT = nc.declare_dram_parameter("wvT", [D, DH], BF16, isOutput=False)
    woT = nc.declare_dram_parameter("woT", [DH, D], BF16, isOutput=False)
    bq_d = nc.declare_dram_parameter("bq", [DH], F32, isOutput=False)
    bk_d = nc.declare_dram_parameter("bk", [DH], F32, isOutput=False)
    bv_d = nc.declare_dram_parameter("bv", [1, DH], F32, isOutput=False)
    bo_d = nc.declare_dram_parameter("bo", [1, D], F32, isOutput=False)
    gam_d = nc.declare_dram_parameter("gamma", [1, D], F32, isOutput=False)
    bet_d = nc.declare_dram_parameter("beta", [1, D], F32, isOutput=False)
    qres_d = nc.declare_dram_parameter("q_res", [S // 2, D], F32, isOutput=False)
    out_d = nc.declare_dram_parameter("out", [S // 2, D], F32, isOutput=True)

    # collective bounce buffers (one pair per 512-token chunk) + recip bounce
    cc_in = [nc.dram_tensor(f"cc_in{i}", [CH, D], BF16) for i in range(NCH)]
    cc_out = [nc.dram_tensor(f"cc_out{i}", [CH // 2, D], BF16) for i in range(NCH)]
    rec_dram = nc.dram_tensor("rec_dram", [QC * 4 * 2, 1024], F32)

    groups = [[0, 1], [2, 3], [4, 5], [6, 7]]

    with tile.TileContext(nc) as tc:
        with tc.tile_pool(name="pers", bufs=1) as pers:
            QT = pers.tile([P, 4, S], BF16, tag="QT")
            KT = pers.tile([P, 4, S], BF16, tag="KT")
            VO = pers.tile([P, KC, HL, DK + 1], BF16, tag="VO")
            ATT = pers.tile([P, 4, S], BF16, tag="ATT")
            WO = pers.tile([P, 4, D], BF16, tag="WO")
            BQK = pers.tile([P, 8], F32, tag="BQK")  # cols 0-3 bq, 4-7 bk

            nc.gpsimd.memset(VO[:, :, :, DK : DK + 1], 1.0)

            # ---------------- projections ----------------
            with tc.tile_pool(name="inp", bufs=1) as inp:
                XV = inp.tile([P, 8, S], BF16, tag="XV")
                XK = inp.tile([P, 8, S], BF16, tag="XK")
                XQ = inp.tile([P, 8, S], BF16, tag="XQ")
                WV = inp.tile([P, 8, DH], BF16, tag="WV")
                WK = inp.tile([P, 8, DH], BF16, tag="WK")
                WQ = inp.tile([P, 8, DH], BF16, tag="WQ")
                BVB = inp.tile([P, DH], F32, tag="BVB")

                # DMA order = consumption order: V (chunked, kc-major), K, Q
                nc.sync.dma_start(
                    out=WV, in_=wvT.ap().rearrange("(c p) n -> p c n", p=P)
                )
                nc.sync.dma_start(out=BVB, in_=bv_d.ap().to_broadcast([P, DH]))
                for kc in range(8):
                    nc.sync.dma_start(
                        out=XV[:, kc, :], in_=xvT.ap()[kc * P : (kc + 1) * P, :]
                    )
                nc.sync.dma_start(
                    out=WK, in_=wkT.ap().rearrange("(c p) n -> p c n", p=P)
                )
                nc.sync.dma_start(
                    out=XK, in_=xkT.ap().rearrange("(c p) s -> p c s", p=P)
                )
                nc.sync.dma_start(
                    out=WQ, in_=wqT.ap().rearrange("(c p) n -> p c n", p=P)
                )
                nc.sync.dma_start(
                    out=XQ, in_=xqT.ap().rearrange("(c p) s -> p c s", p=P)
                )
                nc.sync.dma_start(
                    out=BQK[:, 0:4], in_=bq_d.ap().rearrange("(c p) -> p c", p=P)
                )
                nc.sync.dma_start(
                    out=BQK[:, 4:8], in_=bk_d.ap().rearrange("(c p) -> p c", p=P)
                )
                nc.sync.dma_start(out=WO, in_=woT.ap().rearrange("(c p) d -> p c d", p=P))

                # V = v @ Wv.T + bv: kc-outer waves of 8 token-chunks so the
                # PE starts on XV chunk 0 while later chunks still stream in.
                with tc.tile_pool(name="ppv", bufs=1, space="PSUM") as ppv:
                    for w in range(2):
                        psv = [
                            ppv.tile([P, DH], F32, tag="vw", bufs=8, name=f"psv{w}_{t}")
                            for t in range(8)
                        ]
                        for kc in range(8):
                            for t in range(8):
                                tokc = w * 8 + t
                                nc.tensor.matmul(
                                    psv[t],
                                    lhsT=XV[:, kc, tokc * P : (tokc + 1) * P],
                                    rhs=WV[:, kc, :],
                                    start=(kc == 0),
                                    stop=(kc == 7),
                                )
                        for t in range(8):
                            tokc = w * 8 + t
                            nc.vector.tensor_tensor(
                                VO[:, tokc, :, 0:DK],
                                psv[t].rearrange("p (h d) -> p h d", h=HL),
                                BVB.rearrange("p (h d) -> p h d", h=HL),
                                OP.add,
                            )

                # K^T / Q^T = W @ x^T + b (d_out on partitions)
                with tc.tile_pool(name="ppq", bufs=1, space="PSUM") as ppq:
                    for WX, XX, outT, bcol in ((WK, XK, KT, 4), (WQ, XQ, QT, 0)):
                        for mc in range(4):
                            for nt in range(2):
                                psq = ppq.tile([P, 1024], F32, tag="psq", bufs=2)
                                for kc in range(8):
                                    for half in range(2):
                                        nc.tensor.matmul(
                                            psq[:, half * 512 : (half + 1) * 512],
                                            lhsT=WX[:, kc, mc * P : (mc + 1) * P],
                                            rhs=XX[
                                                :,
                                                kc,
                                                nt * 1024
                                                + half * 512 : nt * 1024
                                                + (half + 1) * 512,
                                            ],
                                            start=(kc == 0),
                                            stop=(kc == 7),
                                        )
                                nc.vector.tensor_scalar_add(
                                    out=outT[:, mc, nt * 1024 : (nt + 1) * 1024],
                                    in0=psq,
                                    scalar1=BQK[:, bcol + mc : bcol + mc + 1],
                                )

            # ---------------- attention + fc + reduce-scatter ----------------
            with (
                tc.tile_pool(name="attp", bufs=1) as attp,
                tc.tile_pool(name="late", bufs=1) as late,
                tc.tile_pool(name="ps", bufs=2, space="PSUM") as ps,
            ):
                GAM = late.tile([P, D], F32, tag="GAM")
                BET = late.tile([P, D], F32, tag="BET")
                BO = late.tile([P, D], F32, tag="BO")
                nc.sync.dma_start(out=GAM, in_=gam_d.ap().to_broadcast([P, D]))
                nc.sync.dma_start(out=BET, in_=bet_d.ap().to_broadcast([P, D]))
                nc.sync.dma_start(out=BO, in_=bo_d.ap().to_broadcast([P, D]))

                EXPA = attp.tile([P, NRING, 1024], BF16, tag="expA")
                EXPB = attp.tile([P, NRING, 1024], BF16, tag="expB")

                def fc_chunk(c):
                    # fc partials for tokens [c*512, (c+1)*512), then bf16 RS
                    for tokc in range(4):
                        tabs = c * 4 + tokc
                        psf = ps.tile([P, D], F32, tag="sc")
                        for dinc in range(4):
                            for half in range(2):
                                nc.tensor.matmul(
                                    psf[:, half * 512 : (half + 1) * 512],
                                    lhsT=ATT[:, dinc, tabs * P : (tabs + 1) * P],
                                    rhs=WO[:, dinc, half * 512 : (half + 1) * 512],
                                    start=(dinc == 0),
                                    stop=(dinc == 3),
                                )
                        fcs = late.tile([P, D], BF16, tag="fcs", bufs=2)
                        nc.scalar.activation(out=fcs, in_=psf, func=AF.Copy)
                        nc.sync.dma_start(
                            out=cc_in[c][tokc * P : (tokc + 1) * P, :], in_=fcs
                        )
                    nc.gpsimd.collective_compute(
                        "ReduceScatter",
                        OP.add,
                        replica_groups=groups,
                        ins=[cc_in[c].ap().opt()],
                        outs=[cc_out[c].ap().opt()],
                    )
                    # prefetch this core's RS result (2 blocks of 128 tokens)
                    # and the matching residual rows
                    for blk in range(2):
                        j = 2 * c + blk
                        xtb = late.tile([P, D], BF16, tag="xtb", bufs=8)
                        nc.sync.dma_start(
                            out=xtb, in_=cc_out[c][blk * P : (blk + 1) * P, :]
                        )
                        qr = late.tile([P, D], F32, tag="qr", bufs=4)
                        nc.sync.dma_start(out=qr, in_=qres_d[j * P : (j + 1) * P, :])
                        _XTB[j] = (xtb, qr)

                def epilogue_block(c, blk):
                    # +bo +residual, LayerNorm, write out for 128 tokens
                    j = 2 * c + blk
                    xtb, qr = _XTB[j]
                    xt = late.tile([P, D], F32, tag="xt", bufs=2)
                    nc.vector.tensor_tensor(xt, xtb, BO, OP.add)
                    nc.vector.tensor_tensor(xt, xt, qr, OP.add)
                    st = late.tile([P, 2, 6], F32, tag="st", bufs=2)
                    nc.vector.bn_stats(st[:, 0, :], xt[:, 0:512])
                    nc.vector.bn_stats(st[:, 1, :], xt[:, 512:1024])
                    mv = late.tile([P, 2], F32, tag="mv", bufs=2)
                    nc.vector.bn_aggr(mv, st)
                    rst = late.tile([P, 1], F32, tag="rst", bufs=2)
                    nc.vector.tensor_scalar_add(
                        out=rst, in0=mv[:, 1:2], scalar1=LN_EPS
                    )
                    nc.vector.reciprocal(rst, rst)
                    nc.scalar.activation(out=rst, in_=rst, func=AF.Sqrt)
                    xn = late.tile([P, D], F32, tag="xn", bufs=2)
                    nc.vector.tensor_scalar(
                        out=xn,
                        in0=xt,
                        scalar1=mv[:, 0:1],
                        scalar2=rst,
                        op0=OP.subtract,
                        op1=OP.mult,
                    )
                    nc.gpsimd.tensor_tensor(xn, xn, GAM, OP.mult)
                    nc.gpsimd.tensor_tensor(xn, xn, BET, OP.add)
                    nc.sync.dma_start(
                        out=out_d[j * P : (j + 1) * P, :], in_=xn
                    )

                _XTB = [None] * 8

                for qc in range(QC):
                    qlo = qc * 1024
                    for hp in range(4):
                        pvA = ps.tile([DK + 1, 1024], F32, tag="pv")
                        pvB = ps.tile([DK + 1, 1024], F32, tag="pv")
                        for kc in range(KC):
                            klo = kc * P
                            r = kc % NRING
                            psc = [None, None]
                            for hb in range(2):
                                plo = hb * 64
                                sc = ps.tile([P, 1024], F32, tag="sc")
                                psc[hb] = sc
                                for half in range(2):
                                    nc.tensor.matmul(
                                        sc[:, half * 512 : (half + 1) * 512],
                                        lhsT=KT[plo : plo + 64, hp, klo : klo + P],
                                        rhs=QT[
                                            plo : plo + 64,
                                            hp,
                                            qlo + half * 512 : qlo + (half + 1) * 512,
                                        ],
                                    )
                            nc.scalar.activation(
                                out=EXPA[:, r, :], in_=psc[0], func=AF.Exp, scale=SCALE
                            )
                            nc.scalar.activation(
                                out=EXPB[:, r, :], in_=psc[1], func=AF.Exp, scale=SCALE
                            )
                            for expT, pv in ((EXPA, pvA), (EXPB, pvB)):
                                for half in range(2):
                                    nc.tensor.matmul(
                                        pv[:, half * 512 : (half + 1) * 512],
                                        lhsT=VO[:, kc, 2 * hp + (0 if expT is EXPA else 1), :],
                                        rhs=expT[:, r, half * 512 : (half + 1) * 512],
                                        start=(kc == 0),
                                        stop=(kc == KC - 1),
                                    )
                        # hp boundary: reciprocal of the softmax sums straight
                        # out of PSUM row 64, SBUF->SBUF broadcast, drain
                        # att_out^T into ATT, then normalize
                        ridx = (qc * 4 + hp) * 2
                        SUa = attp.tile([1, 1024], F32, tag="SUa", bufs=2)
                        SUb = attp.tile([1, 1024], F32, tag="SUb", bufs=2)
                        # order chosen so pvA/pvB free as early as possible:
                        # DVE does recipA, drainA, recipB; ACT drains head B
                        # into its natural idle slot at the hp boundary.
                        nc.vector.reciprocal(SUa, pvA[DK : DK + 1, :])
                        nc.vector.tensor_copy(
                            ATT[0:DK, hp, qlo : qlo + 1024], pvA[0:DK, :]
                        )
                        nc.vector.reciprocal(SUb, pvB[DK : DK + 1, :])
                        tmpB = attp.tile([DK, 1024], BF16, tag="tmpB", bufs=2)
                        nc.scalar.activation(out=tmpB, in_=pvB[0:DK, :], func=AF.Copy)
                        nc.gpsimd.dma_start(
                            out=rec_dram[ridx : ridx + 1, :], in_=SUa
                        )
                        nc.gpsimd.dma_start(
                            out=rec_dram[ridx + 1 : ridx + 2, :], in_=SUb
                        )
                        rb = attp.tile([P, 1024], F32, tag="rb", bufs=2)
                        nc.gpsimd.dma_start(
                            out=rb[0:DK, :],
                            in_=rec_dram[ridx : ridx + 1, :].to_broadcast([DK, 1024]),
                        )
                        nc.gpsimd.dma_start(
                            out=rb[DK:P, :],
                            in_=rec_dram[ridx + 1 : ridx + 2, :].to_broadcast(
                                [DK, 1024]
                            ),
                        )
                        nc.gpsimd.dma_start(
                            out=ATT[DK:P, hp, qlo : qlo + 1024], in_=tmpB
                        )
                        nc.vector.tensor_tensor(
                            ATT[:, hp, qlo : qlo + 1024],
                            ATT[:, hp, qlo : qlo + 1024],
                            rb,
                            OP.mult,
                        )
                    # fc + ReduceScatter for this q-half's two 512-token chunks
                    fc_chunk(2 * qc)
                    fc_chunk(2 * qc + 1)

                # LN epilogues: chunks 0/1 RS completed during qc=1 attention;
                # their DVE work overlaps the fc matmuls of chunks 2/3.
                for c in range(NCH):
                    for blk in range(2):
                        epilogue_block(c, blk)

    nc.compile()
    return nc


def _bf16(a):
    return np.ascontiguousarray(a).astype(ml_dtypes.bfloat16)


def kernel(q, k, v, Wq, bq, Wk, bk, Wv, bv, Wo, bo, gamma, beta, _trace=False):
    global _NC_CACHE
    q = np.asarray(q, np.float32)
    k = np.asarray(k, np.float32)
    v = np.asarray(v, np.float32)
    Wq, Wk, Wv, Wo = (np.asarray(w, np.float32) for w in (Wq, Wk, Wv, Wo))
    bq, bk, bv, bo = (np.asarray(x, np.float32) for x in (bq, bk, bv, bo))
    gamma = np.asarray(gamma, np.float32)
    beta = np.asarray(beta, np.float32)

    in_maps = []
    for c in range(8):
        b, g = divmod(c, 2)
        sl = slice(g * DH, (g + 1) * DH)
        # tokens this core keeps after the chunked pair ReduceScatter:
        # for each 512-token chunk cidx, rows [512c+256g, 512c+256g+256)
        qres = np.concatenate(
            [q[b, 512 * ci + 256 * g : 512 * ci + 256 * g + 256] for ci in range(NCH)]
        )
        in_maps.append(
            {
                "xqT": _bf16(q[b].T),
                "xkT": _bf16(k[b].T),
                "xvT": _bf16(v[b].T),
                "wqT": _bf16(Wq[sl, :].T),
                "wkT": _bf16(Wk[sl, :].T),
                "wvT": _bf16(Wv[sl, :].T),
                "woT": _bf16(Wo[:, sl].T),
                "bq": bq[sl].copy(),
                "bk": bk[sl].copy(),
                "bv": bv[sl].reshape(1, DH).copy(),
                "bo": bo.reshape(1, D).copy(),
                "gamma": gamma.reshape(1, D).copy(),
                "beta": beta.reshape(1, D).copy(),
                "q_res": np.ascontiguousarray(qres),
            }
        )

    if _NC_CACHE is None:
        _NC_CACHE = build_nc()
    nc = _NC_CACHE

    kw = {}
    if _trace:
        import tempfile

        kw = dict(trace=True, tmpdir=tempfile.mkdtemp(prefix="mha_trace_"))
    res = run_bass_kernel_spmd(nc, in_maps, list(range(8)), **kw)
    global _LAST_RES
    _LAST_RES = res

    out = np.empty((B, S, D), np.float32)
    for c in range(8):
        b, g = divmod(c, 2)
        r = res.results[c]["out"]
        for ci in range(NCH):
            out[b, 512 * ci + 256 * g : 512 * ci + 256 * g + 256] = r[
                256 * ci : 256 * ci + 256
            ]

    if _trace:
        kernel._last = res  # stash for test harness
    return out


# revision 20
# speedup vs baseline: 1.1809x; 1.1809x over previous
"""MultiHeadAttention (dense, B=4 S=2048 D=1024 H=16) + residual + LayerNorm
on 8 Trainium2 NeuronCores.

Sharding: core c handles batch b=c//2 and head group g=c%2 (8 of 16 heads),
all 2048 query tokens. The output projection is row-parallel over d_in; a
pair-local bf16 ReduceScatter (cores 2b, 2b+1) sums the two half-head partial
fc outputs per 512-token chunk (4 chunks), after which each core does
bias+residual+LayerNorm for its 256-token half of each chunk (1024 total).

Device layouts (per core):
  QT/KT: [128 part, 4 chunk, 2048 tok] bf16  = projected Q^T / K^T (d_out on
         partitions; head h lives at chunk h//2, partitions 64*(h%2)..+64)
  VO:    [128 part, 16 kchunk, 8 head, 65] bf16 = V rows with a ones column
         appended (col 64) so att@V also yields softmax denominators
  scores are computed transposed ([k, q]) so exp output feeds att@V directly;
  att@V runs V-stationary producing att_out^T [65, q] whose row 64 is the
  softmax sum; normalization multiplies by a broadcast reciprocal.

Pipeline: V-projection runs kc-outer waves over 8 PSUM banks so the PE
starts as soon as the first XV/WV chunks land; per-hp normalization and the
LayerNorm epilogues of finished ReduceScatter chunks are interleaved into
later attention hp iterations so the tensor engine never waits on them.
"""

import numpy as np
import ml_dtypes

import concourse.bass as bass
import concourse.mybir as mybir
import concourse.tile as tile
from concourse import bacc
from concourse.bass_utils import run_bass_kernel_spmd

BF16 = mybir.dt.bfloat16
F32 = mybir.dt.float32
AF = mybir.ActivationFunctionType
OP = mybir.AluOpType

B = 4
S = 2048  # sequence length
D = 1024  # d_model
HL = 8  # heads per core
DK = 64  # head dim
DH = HL * DK  # 512 local projection width
P = 128
KC = S // P  # 16 key chunks
QC = 2  # q halves of 1024
NCH = 4  # reduce-scatter chunks of 512 tokens
CH = S // NCH  # 512
LN_EPS = 1e-5
SCALE = 1.0 / 8.0  # 1/sqrt(DK)
NRING = 12  # exp ring depth (in kc steps)

_NC_CACHE = None
_LAST_RES = None


def build_nc():
    nc = bacc.Bacc(
        None, target_bir_lowering=False, num_devices=8, dynamic_dma_scratch_size=2048
    )

    xqT = nc.declare_dram_parameter("xqT", [D, S], BF16, isOutput=False)
    xkT = nc.declare_dram_parameter("xkT", [D, S], BF16, isOutput=False)
    xvT = nc.declare_dram_parameter("xvT", [D, S], BF16, isOutput=False)
    wqT = nc.declare_dram_parameter("wqT", [D, DH], BF16, isOutput=False)
    wkT = nc.declare_dram_parameter("wkT", [D, DH], BF16, isOutput=False)
    wvT = nc.declare_dram_parameter("wvT", [D, DH], BF16, isOutput=False)
    woT = nc.declare_dram_parameter("woT", [DH, D], BF16, isOutput=False)
    bq_d = nc.declare_dram_parameter("bq", [DH], F32, isOutput=False)
    bk_d = nc.declare_dram_parameter("bk", [DH], F32, isOutput=False)
    bv_d = nc.declare_dram_parameter("bv", [1, DH], F32, isOutput=False)
    bo_d = nc.declare_dram_parameter("bo", [1, D], F32, isOutput=False)
    gam_d = nc.declare_dram_parameter("gamma", [1, D], F32, isOutput=False)
    bet_d = nc.declare_dram_parameter("beta", [1, D], F32, isOutput=False)
    qres_d = nc.declare_dram_parameter("q_res", [S // 2, D], F32, isOutput=False)
    out_d = nc.declare_dram_parameter("out", [S // 2, D], F32, isOutput=True)

    # collective bounce buffers (one pair per 512-token chunk) + recip bounce
    cc_in = [nc.dram_tensor(f"cc_in{i}", [CH, D], BF16) for i in range(NCH)]
    cc_out = [nc.dram_tensor(f"cc_out{i}", [CH // 2, D], BF16) for i in range(NCH)]
    rec_dram = nc.dram_tensor("rec_dram", [QC * 4 * 2, 1024], F32)

    groups = [[0, 1], [2, 3], [4, 5], [6, 7]]

    with tile.TileContext(nc) as tc:
        with tc.tile_pool(name="pers", bufs=1) as pers:
            QT = pers.tile([P, 4, S], BF16, tag="QT")
            KT = pers.tile([P, 4, S], BF16, tag="KT")
            VO = pers.tile([P, KC, HL, DK + 1], BF16, tag="VO")
            ATT = pers.tile([P, 4, S], BF16, tag="ATT")
            WO = pers.tile([P, 4, D], BF16, tag="WO")
            BQK = pers.tile([P, 8], F32, tag="BQK")  # cols 0-3 bq, 4-7 bk

            nc.gpsimd.memset(VO[:, :, :, DK : DK + 1], 1.0)

            # ---------------- projections ----------------
            with tc.tile_pool(name="inp", bufs=1) as inp:
                XV = inp.tile([P, 8, S], BF16, tag="XV")
                XK = inp.tile([P, 8, S], BF16, tag="XK")
                XQ = inp.tile([P, 8, S], BF16, tag="XQ")
                WV = inp.tile([P, 8, DH], BF16, tag="WV")
                WK = inp.tile([P, 8, DH], BF16, tag="WK")
                WQ = inp.tile([P, 8, DH], BF16, tag="WQ")
                BVB = inp.tile([P, DH], F32, tag="BVB")

                # DMA order = consumption order: V (chunked, kc-major), K, Q
                nc.sync.dma_start(
                    out=WV, in_=wvT.ap().rearrange("(c p) n -> p c n", p=P)
                )
                nc.sync.dma_start(out=BVB, in_=bv_d.ap().to_broadcast([P, DH]))
                for kc in range(8):
                    nc.sync.dma_start(
                        out=XV[:, kc, :], in_=xvT.ap()[kc * P : (kc + 1) * P, :]
                    )
                nc.sync.dma_start(
                    out=WK, in_=wkT.ap().rearrange("(c p) n -> p c n", p=P)
                )
                nc.sync.dma_start(
                    out=XK, in_=xkT.ap().rearrange("(c p) s -> p c s", p=P)
                )
                nc.sync.dma_start(
                    out=WQ, in_=wqT.ap().rearrange("(c p) n -> p c n", p=P)
                )
                nc.sync.dma_start(
                    out=XQ, in_=xqT.ap().rearrange("(c p) s -> p c s", p=P)
                )
                nc.sync.dma_start(
                    out=BQK[:, 0:4], in_=bq_d.ap().rearrange("(c p) -> p c", p=P)
                )
                nc.sync.dma_start(
                    out=BQK[:, 4:8], in_=bk_d.ap().rearrange("(c p) -> p c", p=P)
                )
                nc.sync.dma_start(out=WO, in_=woT.ap().rearrange("(c p) d -> p c d", p=P))

                # V = v @ Wv.T + bv: kc-outer waves of 8 token-chunks so the
                # PE starts on XV chunk 0 while later chunks still stream in.
                with tc.tile_pool(name="ppv", bufs=1, space="PSUM") as ppv:
                    for w in range(2):
                        psv = [
                            ppv.tile([P, DH], F32, tag="vw", bufs=8, name=f"psv{w}_{t}")
                            for t in range(8)
                        ]
                        for kc in range(8):
                            for t in range(8):
                                tokc = w * 8 + t
                                nc.tensor.matmul(
                                    psv[t],
                                    lhsT=XV[:, kc, tokc * P : (tokc + 1) * P],
                                    rhs=WV[:, kc, :],
                                    start=(kc == 0),
                                    stop=(kc == 7),
                                )
                        for t in range(8):
                            tokc = w * 8 + t
                            nc.vector.tensor_tensor(
                                VO[:, tokc, :, 0:DK],
                                psv[t].rearrange("p (h d) -> p h d", h=HL),
                                BVB.rearrange("p (h d) -> p h d", h=HL),
                                OP.add,
                            )

                # K^T / Q^T = W @ x^T + b (d_out on partitions)
                with tc.tile_pool(name="ppq", bufs=1, space="PSUM") as ppq:
                    for WX, XX, outT, bcol in ((WK, XK, KT, 4), (WQ, XQ, QT, 0)):
                        for mc in range(4):
                            for nt in range(2):
                                psq = ppq.tile([P, 1024], F32, tag="psq", bufs=2)
                                for kc in range(8):
                                    for half in range(2):
                                        nc.tensor.matmul(
                                            psq[:, half * 512 : (half + 1) * 512],
                                            lhsT=WX[:, kc, mc * P : (mc + 1) * P],
                                            rhs=XX[
                                                :,
                                                kc,
                                                nt * 1024
                                                + half * 512 : nt * 1024
                                                + (half + 1) * 512,
                                            ],
                                            start=(kc == 0),
                                            stop=(kc == 7),
                                        )
                                nc.vector.tensor_scalar_add(
                                    out=outT[:, mc, nt * 1024 : (nt + 1) * 1024],
                                    in0=psq,
                                    scalar1=BQK[:, bcol + mc : bcol + mc + 1],
                                )

            # ---------------- attention + fc + reduce-scatter ----------------
            with (
                tc.tile_pool(name="attp", bufs=1) as attp,
                tc.tile_pool(name="late", bufs=1) as late,
                tc.tile_pool(name="ps", bufs=2, space="PSUM") as ps,
            ):
                GAM = late.tile([P, D], F32, tag="GAM")
                BET = late.tile([P, D], F32, tag="BET")
                BO = late.tile([P, D], F32, tag="BO")
                nc.sync.dma_start(out=GAM, in_=gam_d.ap().to_broadcast([P, D]))
                nc.sync.dma_start(out=BET, in_=bet_d.ap().to_broadcast([P, D]))
                nc.sync.dma_start(out=BO, in_=bo_d.ap().to_broadcast([P, D]))

                EXPA = attp.tile([P, NRING, 1024], BF16, tag="expA")
                EXPB = attp.tile([P, NRING, 1024], BF16, tag="expB")

                def fc_chunk(c):
                    # fc partials for tokens [c*512, (c+1)*512), then bf16 RS
                    for tokc in range(4):
                        tabs = c * 4 + tokc
                        psf = ps.tile([P, D], F32, tag="sc")
                        for dinc in range(4):
                            for half in range(2):
                                nc.tensor.matmul(
                                    psf[:, half * 512 : (half + 1) * 512],
                                    lhsT=ATT[:, dinc, tabs * P : (tabs + 1) * P],
                                    rhs=WO[:, dinc, half * 512 : (half + 1) * 512],
                                    start=(dinc == 0),
                                    stop=(dinc == 3),
                                )
                        fcs = late.tile([P, D], BF16, tag="fcs", bufs=2)
                        nc.scalar.activation(out=fcs, in_=psf, func=AF.Copy)
                        nc.sync.dma_start(
                            out=cc_in[c][tokc * P : (tokc + 1) * P, :], in_=fcs
                        )
                    nc.gpsimd.collective_compute(
                        "ReduceScatter",
                        OP.add,
                        replica_groups=groups,
                        ins=[cc_in[c].ap().opt()],
                        outs=[cc_out[c].ap().opt()],
                    )
                    # prefetch this core's RS result (2 blocks of 128 tokens)
                    # and the matching residual rows
                    for blk in range(2):
                        j = 2 * c + blk
                        xtb = late.tile([P, D], BF16, tag="xtb", bufs=4)
                        nc.sync.dma_start(
                            out=xtb, in_=cc_out[c][blk * P : (blk + 1) * P, :]
                        )
                        qr = late.tile([P, D], F32, tag="qr", bufs=4)
                        nc.sync.dma_start(out=qr, in_=qres_d[j * P : (j + 1) * P, :])
                        _XTB[j] = (xtb, qr)

                def epilogue_block(c, blk):
                    # +bo +residual, LayerNorm, write out for 128 tokens
                    j = 2 * c + blk
                    xtb, qr = _XTB[j]
                    xt = late.tile([P, D], F32, tag="xt", bufs=2)
                    nc.vector.tensor_tensor(xt, xtb, BO, OP.add)
                    nc.vector.tensor_tensor(xt, xt, qr, OP.add)
                    st = late.tile([P, 2, 6], F32, tag="st", bufs=2)
                    nc.vector.bn_stats(st[:, 0, :], xt[:, 0:512])
                    nc.vector.bn_stats(st[:, 1, :], xt[:, 512:1024])
                    mv = late.tile([P, 2], F32, tag="mv", bufs=2)
                    nc.vector.bn_aggr(mv, st)
                    rst = late.tile([P, 1], F32, tag="rst", bufs=2)
                    nc.vector.tensor_scalar_add(
                        out=rst, in0=mv[:, 1:2], scalar1=LN_EPS
                    )
                    nc.vector.reciprocal(rst, rst)
                    nc.scalar.activation(out=rst, in_=rst, func=AF.Sqrt)
                    xn = late.tile([P, D], F32, tag="xn", bufs=2)
                    nc.vector.tensor_scalar(
                        out=xn,
                        in0=xt,
                        scalar1=mv[:, 0:1],
                        scalar2=rst,
                        op0=OP.subtract,
                        op1=OP.mult,
                    )
                    nc.gpsimd.tensor_tensor(xn, xn, GAM, OP.mult)
                    nc.gpsimd.tensor_tensor(xn, xn, BET, OP.add)
                    nc.sync.dma_start(
                        out=out_d[j * P : (j + 1) * P, :], in_=xn
                    )

                _XTB = [None] * 8

                for qc in range(QC):
                    qlo = qc * 1024
                    for hp in range(4):
                        pvA = ps.tile([DK + 1, 1024], F32, tag="pv")
                        pvB = ps.tile([DK + 1, 1024], F32, tag="pv")
                        for kc in range(KC):
                            klo = kc * P
                            r = kc % NRING
                            psc = [None, None]
                            for hb in range(2):
                                plo = hb * 64
                                sc = ps.tile([P, 1024], F32, tag="sc")
                                psc[hb] = sc
                                for half in range(2):
                                    nc.tensor.matmul(
                                        sc[:, half * 512 : (half + 1) * 512],
                                        lhsT=KT[plo : plo + 64, hp, klo : klo + P],
                                        rhs=QT[
                                            plo : plo + 64,
                                            hp,
                                            qlo + half * 512 : qlo + (half + 1) * 512,
                                        ],
                                    )
                            nc.scalar.activation(
                                out=EXPA[:, r, :], in_=psc[0], func=AF.Exp, scale=SCALE
                            )
                            nc.scalar.activation(
                                out=EXPB[:, r, :], in_=psc[1], func=AF.Exp, scale=SCALE
                            )
                            for expT, pv in ((EXPA, pvA), (EXPB, pvB)):
                                for half in range(2):
                                    nc.tensor.matmul(
                                        pv[:, half * 512 : (half + 1) * 512],
                                        lhsT=VO[:, kc, 2 * hp + (0 if expT is EXPA else 1), :],
                                        rhs=expT[:, r, half * 512 : (half + 1) * 512],
                                        start=(kc == 0),
                                        stop=(kc == KC - 1),
                                    )
                        # hp boundary: reciprocal of the softmax sums straight
                        # out of PSUM row 64, SBUF->SBUF broadcast, drain
                        # att_out^T into ATT, then normalize
                        ridx = (qc * 4 + hp) * 2
                        SUia = attp.tile([1, 1024], F32, tag="SUia", bufs=2)
                        SUib = attp.tile([1, 1024], F32, tag="SUib", bufs=2)
                        SUa = attp.tile([1, 1024], F32, tag="SUa", bufs=2)
                        SUb = attp.tile([1, 1024], F32, tag="SUb", bufs=2)
                        # DVE: stage sums rows to SBUF (cheap), then approx
                        # reciprocal (~5x faster than exact, ~18 correct bits).
                        # ACT drains head B into its idle boundary slot.
                        nc.vector.tensor_copy(SUia, pvA[DK : DK + 1, :])
                        nc.vector.tensor_copy(
                            ATT[0:DK, hp, qlo : qlo + 1024], pvA[0:DK, :]
                        )
                        nc.vector.tensor_copy(SUib, pvB[DK : DK + 1, :])
                        tmpB = attp.tile([DK, 1024], BF16, tag="tmpB", bufs=2)
                        nc.scalar.activation(out=tmpB, in_=pvB[0:DK, :], func=AF.Copy)
                        nc.vector.reciprocal_approx_fast(out=SUa, in_=SUia)
                        nc.vector.reciprocal_approx_fast(out=SUb, in_=SUib)
                        nc.gpsimd.dma_start(
                            out=rec_dram[ridx : ridx + 1, :], in_=SUa
                        )
                        nc.gpsimd.dma_start(
                            out=rec_dram[ridx + 1 : ridx + 2, :], in_=SUb
                        )
                        rb = attp.tile([P, 1024], F32, tag="rb", bufs=2)
                        nc.gpsimd.dma_start(
                            out=rb[0:DK, :],
                            in_=rec_dram[ridx : ridx + 1, :].to_broadcast([DK, 1024]),
                        )
                        nc.gpsimd.dma_start(
                            out=rb[DK:P, :],
                            in_=rec_dram[ridx + 1 : ridx + 2, :].to_broadcast(
                                [DK, 1024]
                            ),
                        )
                        nc.gpsimd.dma_start(
                            out=ATT[DK:P, hp, qlo : qlo + 1024], in_=tmpB
                        )
                        nc.vector.tensor_tensor(
                            ATT[:, hp, qlo : qlo + 1024],
                            ATT[:, hp, qlo : qlo + 1024],
                            rb,
                            OP.mult,
                        )
                    # fc + ReduceScatter for this q-half's two 512-token chunks
                    fc_chunk(2 * qc)
                    fc_chunk(2 * qc + 1)

                # LN epilogues: chunks 0/1 RS completed during qc=1 attention;
                # their DVE work overlaps the fc matmuls of chunks 2/3.
                for c in range(NCH):
                    for blk in range(2):
                        epilogue_block(c, blk)

    nc.compile()
    return nc


def _bf16(a):
    return np.ascontiguousarray(a).astype(ml_dtypes.bfloat16)


def kernel(q, k, v, Wq, bq, Wk, bk, Wv, bv, Wo, bo, gamma, beta, _trace=False):
    global _NC_CACHE
    q = np.asarray(q, np.float32)
    k = np.asarray(k, np.float32)
    v = np.asarray(v, np.float32)
    Wq, Wk, Wv, Wo = (np.asarray(w, np.float32) for w in (Wq, Wk, Wv, Wo))
    bq, bk, bv, bo = (np.asarray(x, np.float32) for x in (bq, bk, bv, bo))
    gamma = np.asarray(gamma, np.float32)
    beta = np.asarray(beta, np.float32)

    in_maps = []
    for c in range(8):
        b, g = divmod(c, 2)
        sl = slice(g * DH, (g + 1) * DH)
        # tokens this core keeps after the chunked pair ReduceScatter:
        # for each 512-token chunk cidx, rows [512c+256g, 512c+256g+256)
        qres = np.concatenate(
            [q[b, 512 * ci + 256 * g : 512 * ci + 256 * g + 256] for ci in range(NCH)]
        )
        in_maps.append(
            {
                "xqT": _bf16(q[b].T),
                "xkT": _bf16(k[b].T),
                "xvT": _bf16(v[b].T),
                "wqT": _bf16(Wq[sl, :].T),
                "wkT": _bf16(Wk[sl, :].T),
                "wvT": _bf16(Wv[sl, :].T),
                "woT": _bf16(Wo[:, sl].T),
                "bq": bq[sl].copy(),
                "bk": bk[sl].copy(),
                "bv": bv[sl].reshape(1, DH).copy(),
                "bo": bo.reshape(1, D).copy(),
                "gamma": gamma.reshape(1, D).copy(),
                "beta": beta.reshape(1, D).copy(),
                "q_res": np.ascontiguousarray(qres),
            }
        )

    if _NC_CACHE is None:
        _NC_CACHE = build_nc()
    nc = _NC_CACHE

    kw = {}
    if _trace:
        import tempfile

        kw = dict(trace=True, tmpdir=tempfile.mkdtemp(prefix="mha_trace_"))
    res = run_bass_kernel_spmd(nc, in_maps, list(range(8)), **kw)
    global _LAST_RES
    _LAST_RES = res

    out = np.empty((B, S, D), np.float32)
    for c in range(8):
        b, g = divmod(c, 2)
        r = res.results[c]["out"]
        for ci in range(NCH):
            out[b, 512 * ci + 256 * g : 512 * ci + 256 * g + 256] = r[
                256 * ci : 256 * ci + 256
            ]

    if _trace:
        kernel._last = res  # stash for test harness
    return out
